# revision 22
# baseline (speedup 1.0000x reference)
"""FPLPGCN (2x GCNConv feature prop + 10x label prop + fuse) on 8 trn2 cores.

Strategy (graph/data parallel, v2):
- Nodes sorted by (mask, in-degree), striped round-robin across 8 cores.
  Masked (label-overwritten) nodes occupy low local rows; unmasked high rows
  [B*, NPAD).  Each core owns NPAD=12544 rows (44 fake pad rows).
- GCN refactor: out[n] = dinv[n]*(sum_{e->n} u'[src] + u'[n]) + b with
  u' = dinv*(z @ W).  Self terms are computed locally (no self edges).
- Rounds r0/r1 use a paired table [u_feat 64 | t_lbl 32 | pad 32] bf16 so one
  gather serves both chains.  The label chain postpones its 32x32 weight to
  AFTER aggregation, so the label table holds t = dinv*xl directly.
- Label-only rounds 3..10 gather ONLY unmasked-src -> unmasked-dst edges.
  Masked sources always contribute c[n] = sum dinv*y[src] (constant): c is
  aggregated once at startup from a dinv*y table (layC).
- Aggregation: dense-packed dma_gather slots (256B elems); per 128-slot
  column a one-hot (dst-partition) matrix streamed from DRAM routes messages
  into per-window PSUM via PE matmul-accumulation.  Post-processing
  (self terms, weights, mask overwrite, bounce prep) is batched into
  per-span 3D-broadcast vector ops.
"""

import sys

sys.path.insert(0, "/opt/trn_rl_repo")

import numpy as np

NC = 8
P = 128
NPAD = 12544           # local rows per core (98 windows of 128)
NWIN = NPAD // P       # 98
TABROWS = NC * NPAD    # 100352
CHP = TABROWS // 4     # 25088: paired chunk (row stride 256B)
IN_DIM, HID, OUT, DW = 128, 64, 32, 64
PAIR_F = HID + OUT     # 96 useful cols of the 128-wide paired row
NUM_LBL = 10
SPAN_P = 5             # windows per paired span (psum: 5*96*4B = 1920B/bank)
SPAN_L = 10            # windows per label span (psum: 10*32*4B = 1280B)
MAXCOLS = 28           # <=3584 idxs per dma_gather call


def _cdiv(a, b):
    return -(-a // b)


# ----------------------------------------------------------------------------
# host-side index preprocessing (pure index manipulation; no FP math on data)
# ----------------------------------------------------------------------------

def _build_layout(ecore, ewin, epart, grp, idxv, spans):
    """Dense slot layout for one gather addressing scheme."""
    EA = ecore.shape[0]
    key = (ecore * NWIN + ewin) * 4 + grp
    cnt = np.bincount(key, minlength=NC * NWIN * 4).reshape(NC, NWIN, 4)
    ncols = _cdiv(cnt.max(axis=0), 128)                     # [NWIN, 4]

    col_start = np.zeros((NWIN, 4), np.int64)
    region_col0 = np.zeros((len(spans), 4), np.int64)
    region_off16 = np.zeros((len(spans), 4), np.int64)
    col = 0
    off16 = 0
    span_of_w = np.full(NWIN, -1, np.int64)
    for s, (w0, w1) in enumerate(spans):
        span_of_w[w0:w1] = s
        for g in range(4):
            region_col0[s, g] = col
            region_off16[s, g] = off16
            for w in range(w0, w1):
                col_start[w, g] = col
                col += int(ncols[w, g])
            off16 += int((col - region_col0[s, g]) * 8)
    totcols = col
    tot16 = off16

    o = np.argsort(key, kind="stable")
    ks = key[o]
    first = np.searchsorted(ks, ks, side="left")
    pos = np.empty(EA, np.int64)
    pos[o] = np.arange(EA) - first

    ecolg = col_start[ewin, grp] + pos // 128
    part = pos % 128
    es = span_of_w[ewin]
    i_in_region = (ecolg - region_col0[es, grp]) * 128 + part

    idx16 = np.zeros((NC, 16, max(tot16, 8)), np.int16)
    idx16[ecore, i_in_region % 16,
          region_off16[es, grp] + i_in_region // 16] = idxv.astype(np.int16)

    dpart = np.full((NC, 128, max(totcols, 1)), 128.0, np.float32)
    dpart[ecore, part, ecolg] = epart.astype(np.float32)

    return dict(idx16=idx16, dpart=dpart, ncols=ncols, spans=spans,
                region_col0=region_col0, region_off16=region_off16,
                totcols=totcols, tot16=tot16)


def _preprocess(edge_index, mask, n_nodes):
    src = np.ascontiguousarray(edge_index[0]).astype(np.int64)
    dst = np.ascontiguousarray(edge_index[1]).astype(np.int64)
    deg = np.bincount(dst, minlength=n_nodes).astype(np.int64)
    mask = np.ascontiguousarray(mask).astype(bool)
    nfake = TABROWS - n_nodes

    # masked nodes first, then unmasked; degree-sorted within each group
    order = np.lexsort((deg, (~mask).astype(np.int64)))
    rank = np.empty(n_nodes, np.int64)
    rank[order] = np.arange(nfake, TABROWS)
    core_of = rank % NC
    local_of = rank // NC
    trow = core_of * NPAD + local_of

    M = int(mask.sum())
    wU = (nfake + M) // (NC * P)          # first label-updated window
    assert 1 <= wU <= NWIN - 1
    BST = wU * P                          # boundary local row (window aligned)
    RU = NPAD - BST                       # label-table rows per core

    spansP = [(w0, min(w0 + SPAN_P, NWIN)) for w0 in range(0, NWIN, SPAN_P)]
    spansL = [(w0, min(w0 + SPAN_L, NWIN)) for w0 in range(wU, NWIN, SPAN_L)]

    layP = _build_layout(core_of[dst], local_of[dst] // P, local_of[dst] % P,
                         trow[src] // CHP, trow[src] % CHP, spansP)

    selL = (local_of[src] >= BST) & (local_of[dst] >= BST)
    tL = core_of * RU + (local_of - BST)
    sL, dL = src[selL], dst[selL]
    layL = _build_layout(core_of[dL], local_of[dL] // P, local_of[dL] % P,
                         tL[sL] % 4, tL[sL] // 4, spansL)

    selC = (local_of[src] < BST) & (local_of[dst] >= BST)
    tC = core_of * BST + local_of
    sC, dC = src[selC], dst[selC]
    layC = _build_layout(core_of[dC], local_of[dC] // P, local_of[dC] % P,
                         tC[sC] % 4, tC[sC] // 4, spansL)

    meta = dict(order=order, core_of=core_of, local_of=local_of,
                deg=deg, wU=wU, BST=BST, RU=RU,
                layP=layP, layL=layL, layC=layC)
    return meta


def _shard_nodes(arr, core_of, local_of, width, dtype=np.float32):
    n = arr.shape[0]
    out = np.zeros((NC, NPAD, width), dtype)
    a2 = np.asarray(arr, dtype).reshape(n, width)
    out[core_of, local_of] = a2
    return out


# ----------------------------------------------------------------------------
# device program
# ----------------------------------------------------------------------------

def _build(meta, nonzero_b):
    import concourse.bacc as bacc
    import concourse.bass as bass
    import concourse.mybir as mybir
    import concourse.tile as tile
    from concourse.bass import AP

    f32 = mybir.dt.float32
    bf16 = mybir.dt.bfloat16
    layP, layL, layC = meta["layP"], meta["layL"], meta["layC"]
    wU, BST, RU = meta["wU"], meta["BST"], meta["RU"]
    NWU = NWIN - wU

    nc = bacc.Bacc("TRN2", target_bir_lowering=False, debug=False,
                   num_devices=NC, num_swdge_queues=4)

    x_sh = nc.dram_tensor("x_sh", [NPAD, IN_DIM], f32, kind="ExternalInput")
    y_sh = nc.dram_tensor("y_sh", [NPAD, OUT], f32, kind="ExternalInput")
    dw_sh = nc.dram_tensor("dw_sh", [NPAD, DW], f32, kind="ExternalInput")
    mask_sh = nc.dram_tensor("mask_sh", [NPAD, 1], mybir.dt.int8,
                             kind="ExternalInput")
    deg_sh = nc.dram_tensor("deg_sh", [NPAD, 1], mybir.dt.int32,
                            kind="ExternalInput")
    idxP_d = nc.dram_tensor("idxP_d", [P, layP["tot16"]], mybir.dt.int16,
                            kind="ExternalInput")
    idxL_d = nc.dram_tensor("idxL_d", [P, layL["tot16"]], mybir.dt.int16,
                            kind="ExternalInput")
    idxC_d = nc.dram_tensor("idxC_d", [P, layC["tot16"]], mybir.dt.int16,
                            kind="ExternalInput")
    ohP_d = nc.dram_tensor("ohP_d", [P, layP["totcols"] * P], bf16,
                           kind="ExternalInput")
    ohL_d = nc.dram_tensor("ohL_d", [P, layL["totcols"] * P], bf16,
                           kind="ExternalInput")
    ohC_d = nc.dram_tensor("ohC_d", [P, layC["totcols"] * P], bf16,
                           kind="ExternalInput")
    W0_d = nc.dram_tensor("W0", [IN_DIM, HID], f32, kind="ExternalInput")
    W1_d = nc.dram_tensor("W1", [HID, HID], f32, kind="ExternalInput")
    Wl_d = nc.dram_tensor("Wl", [NUM_LBL * OUT, OUT], f32, kind="ExternalInput")
    Wf_d = nc.dram_tensor("Wf", [HID + OUT + DW, OUT], f32, kind="ExternalInput")
    b_d = nc.dram_tensor("b_all", [4, max(HID, OUT) * NUM_LBL], f32,
                         kind="ExternalInput")
    out_sh = nc.dram_tensor("out_sh", [NPAD, OUT], f32, kind="ExternalOutput")

    # internal DRAM
    tabP = [nc.dram_tensor(f"tabP{i}", [TABROWS, P], bf16,
                           addr_space="Shared") for i in range(2)]
    tabL = [nc.dram_tensor(f"tabL{i}", [NC * RU + 8, OUT], bf16,
                           addr_space="Shared") for i in range(2)]
    tabC = nc.dram_tensor("tabC", [NC * BST + 8, OUT], bf16,
                          addr_space="Shared")
    bnP = [nc.dram_tensor(f"bnP{i}", [NPAD, P], bf16) for i in range(2)]
    bnL = [nc.dram_tensor(f"bnL{i}", [RU, OUT], bf16) for i in range(2)]
    bnC = nc.dram_tensor("bnC", [BST, OUT], bf16)

    with tile.TileContext(nc) as tc:
        with tc.tile_pool(name="persist", bufs=1) as pp, \
             tc.tile_pool(name="g", bufs=6) as gp, \
             tc.tile_pool(name="ix", bufs=3) as ixp, \
             tc.tile_pool(name="oh", bufs=3) as ohp, \
             tc.tile_pool(name="wk", bufs=3) as wk, \
             tc.tile_pool(name="ps", bufs=2, space="PSUM") as ps, \
             tc.tile_pool(name="psu", bufs=2, space="PSUM") as psu, \
             tc.tile_pool(name="psw", bufs=4, space="PSUM") as psw:

            # ---- constants / persistent state ----
            W0 = pp.tile([IN_DIM, HID], f32)
            nc.sync.dma_start(out=W0[:], in_=W0_d[:, :])
            W1r = pp.tile([P, HID], f32)
            for a in range(2):
                nc.sync.dma_start(out=W1r[a * HID:(a + 1) * HID, :],
                                  in_=W1_d[:, :])
            Wl4 = pp.tile([P, NUM_LBL * OUT], f32)
            for a in range(3):
                nc.sync.dma_start(
                    out=Wl4[a * OUT:(a + 1) * OUT, :]
                        .rearrange("p (j f) -> p j f", j=NUM_LBL),
                    in_=Wl_d[:, :].rearrange("(j k) f -> k j f", k=OUT))
            Wfa = pp.tile([128, OUT], f32)
            nc.sync.dma_start(out=Wfa[:], in_=Wf_d[0:128, :])
            Wfb = pp.tile([HID + OUT + DW - 128, OUT], f32)
            nc.sync.dma_start(out=Wfb[:], in_=Wf_d[128:, :])
            from concourse.masks import make_identity
            ident = pp.tile([P, P], f32)
            make_identity(nc, ident[:])
            onecol = pp.tile([1, P], f32)
            nc.vector.memset(onecol[:], 1.0)

            yb = pp.tile([P, NWIN * OUT], f32)
            nc.sync.dma_start(
                out=yb[:].rearrange("p (w f) -> p w f", w=NWIN),
                in_=y_sh[:, :].rearrange("(w p) f -> p w f", p=P))
            maskb = pp.tile([P, NWIN], mybir.dt.int8)
            nc.sync.dma_start(
                out=maskb[:], in_=mask_sh[:, 0].rearrange("(w p) -> p w", p=P))
            degb = pp.tile([P, NWIN], mybir.dt.int32)
            nc.sync.dma_start(
                out=degb[:], in_=deg_sh[:, 0].rearrange("(w p) -> p w", p=P))

            degf = pp.tile([P, NWIN], f32)
            nc.vector.tensor_copy(out=degf[:], in_=degb[:])
            recipb = pp.tile([P, NWIN], f32)
            nc.vector.tensor_scalar(out=degf[:], in0=degf[:], scalar1=1.0,
                                    scalar2=None, op0=mybir.AluOpType.add)
            nc.vector.reciprocal(out=recipb[:], in_=degf[:])      # 1/(deg+1)
            dinvb = pp.tile([P, NWIN], f32)
            nc.scalar.sqrt(out=dinvb[:], in_=recipb[:])           # 1/sqrt(deg+1)
            nfake = TABROWS - 100000
            nc.vector.memset(recipb[0:nfake // NC, 0:1], 0.0)
            nc.vector.memset(dinvb[0:nfake // NC, 0:1], 0.0)
            dinvy = pp.tile([P, NWIN * OUT], f32)
            for w in range(NWIN):
                nc.vector.tensor_scalar(
                    out=dinvy[:, w * OUT:(w + 1) * OUT],
                    in0=yb[:, w * OUT:(w + 1) * OUT],
                    scalar1=dinvb[:, w:w + 1], scalar2=None,
                    op0=mybir.AluOpType.mult)

            def bias_tile(row, width):
                bt = pp.tile([P, width], f32, tag=f"bias{row}", name=f"bias{row}")
                brow = pp.tile([1, width], f32, tag=f"brow{row}", name=f"brow{row}")
                nc.sync.dma_start(out=brow[:], in_=b_d[row:row + 1, 0:width])
                pt = ps.tile([P, P], f32, tag="tps", name="biasps")
                nc.tensor.matmul(out=pt[:, 0:width], lhsT=onecol[:],
                                 rhs=brow[:], start=True, stop=True)
                nc.vector.tensor_copy(out=bt[:], in_=pt[:, 0:width])
                return bt

            bias0 = bias_tile(0, HID) if nonzero_b[0] else None
            bias1 = bias_tile(1, HID) if nonzero_b[1] else None
            biasf = bias_tile(3, OUT) if nonzero_b[3] else None

            vF = pp.tile([P, NWIN * HID], f32)      # feature state (h at end)
            vL = pp.tile([P, NWIN * OUT], f32)      # label state xl (plain)
            ulocF = pp.tile([P, NWIN * HID], bf16)  # own feature table rows
            cbuf = pp.tile([P, NWU * OUT], f32)     # masked-src label constant

            # resident index table for the 8 label rounds
            ixL = pp.tile([P, layL["tot16"]], mybir.dt.int16)
            nc.sync.dma_start(out=ixL[:], in_=idxL_d[:, :])

            # vL starts as y (masked windows keep it forever)
            nc.vector.tensor_copy(out=vL[:], in_=yb[:])

            qctr = [0]

            def bc3(t, w0, w1, F):
                """[P, w1-w0, F] view of t[:, w0:w1] with 0-stride inner."""
                sl = t[:, w0:w1]
                return AP(sl.tensor, sl.offset,
                          [sl.ap[0], sl.ap[1], [0, F]])

            # ---- aggregation engine ----
            def stage_agg(lay, tab_in_aps, ix_src, oh_src, F, span_cb):
                """Gather + one-hot matmul segment-sum.

                ix_src: None -> resident (use ix_tile slices); else DRAM tensor.
                span_cb(s, w0, w1, pt, wtot): called once per span after its
                accumulation chain closes; pt[:, (w-w0)*F:...] holds window w's
                aggregate (only valid when wtot[w] > 0).
                """
                ncols = lay["ncols"]
                spans = lay["spans"]
                region_col0 = lay["region_col0"]
                region_off16 = lay["region_off16"]
                for s, (w0, w1) in enumerate(spans):
                    span_off16 = int(region_off16[s, 0])
                    span_cols = int(ncols[w0:w1, :].sum())
                    wtot = {w: int(ncols[w, :].sum()) for w in range(w0, w1)}
                    nw = w1 - w0
                    if span_cols == 0:
                        span_cb(s, w0, w1, None, wtot)
                        continue
                    span_n16 = span_cols * 8
                    if ix_src is None:
                        ixt = ixL
                        ix_base = span_off16
                    else:
                        ixt = ixp.tile([P, span_n16], mybir.dt.int16, tag="ix")
                        nc.sync.dma_start(
                            out=ixt[:],
                            in_=ix_src[:, span_off16:span_off16 + span_n16])
                        ix_base = 0

                    pt = psw.tile([P, nw * F], f32, tag="aggps", name="aggps")
                    kspan = [0]
                    kspan_tot = int(sum(wtot.values()))

                    for g in range(4):
                        rcols = int(ncols[w0:w1, g].sum())
                        if rcols == 0:
                            continue
                        o16 = ix_base + int(region_off16[s, g]) - span_off16
                        rcol0 = int(region_col0[s, g])
                        tiles = []
                        ohts = []
                        for c0 in range(0, rcols, MAXCOLS):
                            c1 = min(c0 + MAXCOLS, rcols)
                            nidx = (c1 - c0) * P
                            gt = gp.tile([P, (c1 - c0) * P], bf16, tag="g")
                            nc.gpsimd.dma_gather(
                                out_ap=gt[:].rearrange("p (s f) -> p s f", f=P),
                                in_ap=tab_in_aps[g],
                                idxs_ap=ixt[:, o16 + c0 * 8:o16 + c1 * 8],
                                num_idxs=nidx, num_idxs_reg=nidx,
                                elem_size=P, queue_num=qctr[0] % 4,
                                single_packet=False)
                            qctr[0] += 1
                            tiles.append(gt)
                            oht = ohp.tile([P, (c1 - c0) * P], bf16, tag="oh")
                            nc.sync.dma_start(
                                out=oht[:],
                                in_=oh_src[:, (rcol0 + c0) * P:(rcol0 + c1) * P])
                            ohts.append(oht)
                        creg = 0
                        for w in range(w0, w1):
                            n = int(ncols[w, g])
                            pslice = pt[:, (w - w0) * F:(w - w0 + 1) * F]
                            for c in range(n):
                                cr = creg + c
                                gt = tiles[cr // MAXCOLS]
                                oht = ohts[cr // MAXCOLS]
                                toff = cr % MAXCOLS
                                ks = kspan[0]
                                nc.tensor.matmul(
                                    out=pslice,
                                    lhsT=oht[:, toff * P:(toff + 1) * P],
                                    rhs=gt[:, toff * P:toff * P + F],
                                    start=(ks == 0),
                                    stop=(ks == kspan_tot - 1))
                                kspan[0] = ks + 1
                            creg += n
                    span_cb(s, w0, w1, pt, wtot)

            def tabP_aps(t):
                return [t[q * CHP:(q + 1) * CHP, :] for q in range(4)]

            def tab4_aps(t, nrows_alloc):
                k4 = (nrows_alloc - 4) // 4
                return [t[q:q + 4 * k4, :].rearrange(
                    "(r k) f -> r (k f)", k=4) for q in range(4)]

            # ---- shared post-processing helpers ----
            def apply_Wl(j, Gblk, w0, nw, dest_cb):
                """dest_cb(w, up_slice_ap) with up = Gblk@W_label[j] per window."""
                for b0 in range(0, nw, 3):
                    b1 = min(b0 + 3, nw)
                    nb = b1 - b0
                    tp = ps.tile([P, P], f32, tag="tps")
                    nc.tensor.transpose(
                        out=tp[0:nb * OUT, :],
                        in_=Gblk[:, b0 * OUT:b1 * OUT], identity=ident[:])
                    vT = wk.tile([P, P], f32, tag="vT")
                    nc.scalar.copy(out=vT[0:nb * OUT, :], in_=tp[0:nb * OUT, :])
                    for a in range(nb):
                        up = psu.tile([P, OUT], f32, tag="ups")
                        nc.tensor.matmul(
                            out=up[:],
                            lhsT=vT[a * OUT:(a + 1) * OUT, :],
                            rhs=Wl4[a * OUT:(a + 1) * OUT,
                                    j * OUT:(j + 1) * OUT],
                            start=True, stop=True)
                        dest_cb(w0 + b0 + a, up[:])

            def fuse_window(w, oblk, ob_off):
                dwt = wk.tile([P, DW], f32, tag="dwt")
                nc.sync.dma_start(out=dwt[:], in_=dw_sh[w * P:(w + 1) * P, :])
                fTa = wk.tile([P, P], f32, tag="fTa")
                fTb = wk.tile([DW - 32, P], f32, tag="fTb")
                tp = ps.tile([P, P], f32, tag="tps")
                nc.tensor.transpose(out=tp[0:HID, :],
                                    in_=vF[:, w * HID:(w + 1) * HID],
                                    identity=ident[:])
                nc.scalar.copy(out=fTa[0:HID, :], in_=tp[0:HID, :])
                tp2 = ps.tile([P, P], f32, tag="tps")
                nc.tensor.transpose(out=tp2[0:OUT, :],
                                    in_=vL[:, w * OUT:(w + 1) * OUT],
                                    identity=ident[:])
                nc.scalar.copy(out=fTa[HID:HID + OUT, :], in_=tp2[0:OUT, :])
                tp3 = ps.tile([P, P], f32, tag="tps")
                nc.tensor.transpose(out=tp3[0:DW, :], in_=dwt[:],
                                    identity=ident[:])
                nc.scalar.copy(out=fTa[HID + OUT:P, :],
                               in_=tp3[0:P - HID - OUT, :])
                nc.scalar.copy(out=fTb[:, :], in_=tp3[P - HID - OUT:DW, :])
                op = psu.tile([P, OUT], f32, tag="ups", name="ops")
                nc.tensor.matmul(out=op[:], lhsT=fTa[:], rhs=Wfa[:],
                                 start=True, stop=False)
                nc.tensor.matmul(out=op[:], lhsT=fTb[:], rhs=Wfb[:],
                                 start=False, stop=True)
                if biasf is not None:
                    nc.vector.tensor_add(out=op[:], in0=op[:], in1=biasf[:])
                nc.scalar.activation(
                    out=oblk[:, ob_off * OUT:(ob_off + 1) * OUT], in_=op[:],
                    func=bass.mybir.ActivationFunctionType.Sigmoid)

            # ======================================================================
            # 1. bnC = bf16(dinvy rows [0, BST)) ; AG_C -> tabC
            # ======================================================================
            CB = 7
            for wb in range(0, wU, CB):
                nwb = min(CB, wU - wb)
                cblk = wk.tile([P, nwb * OUT], bf16, tag="cblk")
                nc.vector.tensor_copy(
                    out=cblk[:], in_=dinvy[:, wb * OUT:(wb + nwb) * OUT])
                nc.sync.dma_start(
                    out=bnC[wb * P:(wb + nwb) * P, :]
                        .rearrange("(w p) f -> p w f", p=P),
                    in_=cblk[:].rearrange("p (w f) -> p w f", w=nwb))
            nc.gpsimd.collective_compute(
                "AllGather", bass.mybir.AluOpType.bypass,
                replica_groups=[list(range(NC))],
                ins=[bnC[:, :].opt()],
                outs=[tabC[0:NC * BST, :].opt()])

            # ======================================================================
            # 2. init tables: [dinv*(x@W0) | dinvy] -> bnP[0] ; AG0
            # ======================================================================
            XB = 4
            for wb in range(0, NWIN, XB):
                nwb = min(XB, NWIN - wb)
                ublk = wk.tile([P, nwb * PAIR_F], bf16, tag="ubx")
                xt4 = wk.tile([P, nwb * IN_DIM], f32, tag="xt")
                nc.sync.dma_start(
                    out=xt4[:].rearrange("p (w f) -> p w f", w=nwb),
                    in_=x_sh[wb * P:(wb + nwb) * P, :]
                        .rearrange("(w p) f -> p w f", p=P))
                nc.vector.tensor_tensor(
                    out=xt4[:].rearrange("p (w f) -> p w f", f=IN_DIM),
                    in0=xt4[:].rearrange("p (w f) -> p w f", f=IN_DIM),
                    in1=bc3(dinvb, wb, wb + nwb, IN_DIM),
                    op=mybir.AluOpType.mult)
                for a in range(nwb):
                    w = wb + a
                    xt = xt4[:, a * IN_DIM:(a + 1) * IN_DIM]
                    tp = ps.tile([P, P], f32, tag="tps")
                    nc.tensor.transpose(out=tp[:], in_=xt, identity=ident[:])
                    vT = wk.tile([P, P], f32, tag="vT")
                    nc.scalar.copy(out=vT[:], in_=tp[:])
                    up = psu.tile([P, HID], f32, tag="ups")
                    nc.tensor.matmul(out=up[:], lhsT=vT[:], rhs=W0[:],
                                     start=True, stop=True)
                    nc.scalar.copy(out=ublk[:, a * PAIR_F:a * PAIR_F + HID],
                                   in_=up[:])
                    nc.scalar.copy(out=ulocF[:, w * HID:(w + 1) * HID],
                                   in_=up[:])
                    nc.vector.tensor_copy(
                        out=ublk[:, a * PAIR_F + HID:(a + 1) * PAIR_F],
                        in_=dinvy[:, w * OUT:(w + 1) * OUT])
                nc.sync.dma_start(
                    out=bnP[0][wb * P:(wb + nwb) * P, 0:PAIR_F]
                        .rearrange("(w p) f -> p w f", p=P),
                    in_=ublk[:].rearrange("p (w f) -> p w f", w=nwb))
            nc.gpsimd.collective_compute(
                "AllGather", bass.mybir.AluOpType.bypass,
                replica_groups=[list(range(NC))],
                ins=[bnP[0][:, :].opt()],
                outs=[tabP[0][0:TABROWS, :].opt()])

            # ======================================================================
            # 3. c aggregation (overlaps r0): cbuf = agg of dinv*y from masked srcs
            # ======================================================================
            def cb_c(s, w0, w1, pt, wtot):
                if pt is not None and all(wtot[w] > 0 for w in range(w0, w1)):
                    nc.vector.tensor_copy(
                        out=cbuf[:, (w0 - wU) * OUT:(w1 - wU) * OUT],
                        in_=pt[:, 0:(w1 - w0) * OUT])
                    return
                for w in range(w0, w1):
                    dst = cbuf[:, (w - wU) * OUT:(w - wU + 1) * OUT]
                    if pt is not None and wtot[w] > 0:
                        nc.vector.tensor_copy(
                            out=dst, in_=pt[:, (w - w0) * OUT:(w - w0 + 1) * OUT])
                    else:
                        nc.vector.memset(dst, 0.0)
            stage_agg(layC, tab4_aps(tabC, NC * BST + 8), idxC_d, ohC_d, OUT,
                      cb_c)

            # ======================================================================
            # 4. paired rounds r0 / r1
            # ======================================================================
            def paired_round(r):
                tab = tabP[r]
                jW = r                       # label weight index for this round

                def cb(s, w0, w1, pt, wtot):
                    nw = w1 - w0
                    Gblk = wk.tile([P, nw * OUT], f32, tag="Gblk")
                    pt3 = pt[:, 0:nw * PAIR_F].rearrange(
                        "p (w f) -> p w f", f=PAIR_F)
                    vF3 = vF[:, w0 * HID:w1 * HID].rearrange(
                        "p (w f) -> p w f", f=HID)
                    nc.vector.tensor_add(
                        out=vF3, in0=pt3[:, :, 0:HID],
                        in1=ulocF[:, w0 * HID:w1 * HID].rearrange(
                            "p (w f) -> p w f", f=HID))
                    sc = recipb if r == 0 else dinvb
                    nc.vector.tensor_tensor(out=vF3, in0=vF3,
                                            in1=bc3(sc, w0, w1, HID),
                                            op=mybir.AluOpType.mult)
                    if r == 0 and bias0 is not None:
                        for w in range(w0, w1):
                            dstF = vF[:, w * HID:(w + 1) * HID]
                            dv = wk.tile([P, HID], f32, tag="dbv")
                            nc.vector.tensor_scalar(out=dv[:], in0=bias0[:],
                                                    scalar1=dinvb[:, w:w + 1],
                                                    scalar2=None,
                                                    op0=mybir.AluOpType.mult)
                            nc.vector.tensor_add(out=dstF, in0=dstF, in1=dv[:])
                    if r == 1 and bias1 is not None:
                        for w in range(w0, w1):
                            dstF = vF[:, w * HID:(w + 1) * HID]
                            nc.vector.tensor_add(out=dstF, in0=dstF,
                                                 in1=bias1[:])
                    # label: G = dinv*(psL + self)
                    G3 = Gblk[:].rearrange("p (w f) -> p w f", f=OUT)
                    vL3 = vL[:, w0 * OUT:w1 * OUT].rearrange(
                        "p (w f) -> p w f", f=OUT)
                    db = bc3(dinvb, w0, w1, OUT)
                    if r == 0:
                        nc.vector.tensor_add(
                            out=G3, in0=pt3[:, :, HID:PAIR_F],
                            in1=dinvy[:, w0 * OUT:w1 * OUT].rearrange(
                                "p (w f) -> p w f", f=OUT))
                    else:
                        nc.vector.tensor_tensor(out=G3, in0=vL3, in1=db,
                                                op=mybir.AluOpType.mult)
                        nc.vector.tensor_add(out=G3, in0=G3,
                                             in1=pt3[:, :, HID:PAIR_F])
                    nc.vector.tensor_tensor(out=G3, in0=G3, in1=db,
                                            op=mybir.AluOpType.mult)

                    def dest(w, upsl):
                        nc.vector.tensor_copy(
                            out=vL[:, w * OUT:(w + 1) * OUT], in_=upsl)
                    apply_Wl(jW, Gblk, w0, nw, dest)
                    nc.vector.copy_predicated(
                        out=vL3, mask=bc3(maskb, w0, w1, OUT),
                        data=yb[:, w0 * OUT:w1 * OUT].rearrange(
                            "p (w f) -> p w f", f=OUT))

                    if r == 0:
                        # build r1 paired table rows for these windows
                        ublk = wk.tile([P, nw * PAIR_F], bf16, tag="ubp")
                        for b0 in range(0, nw, 2):
                            b1 = min(b0 + 2, nw)
                            nb = b1 - b0
                            tp = ps.tile([P, P], f32, tag="tps")
                            nc.tensor.transpose(
                                out=tp[0:nb * HID, :],
                                in_=vF[:, (w0 + b0) * HID:(w0 + b1) * HID],
                                identity=ident[:])
                            vT = wk.tile([P, P], f32, tag="vT")
                            nc.scalar.copy(out=vT[0:nb * HID, :],
                                           in_=tp[0:nb * HID, :])
                            for a in range(nb):
                                w = w0 + b0 + a
                                up = psu.tile([P, HID], f32, tag="ups")
                                nc.tensor.matmul(
                                    out=up[:],
                                    lhsT=vT[a * HID:(a + 1) * HID, :],
                                    rhs=W1r[a * HID:(a + 1) * HID, :],
                                    start=True, stop=True)
                                nc.scalar.copy(
                                    out=ublk[:, (b0 + a) * PAIR_F:
                                             (b0 + a) * PAIR_F + HID],
                                    in_=up[:])
                                nc.scalar.copy(
                                    out=ulocF[:, w * HID:(w + 1) * HID],
                                    in_=up[:])
                        nc.vector.tensor_tensor(
                            out=ublk[:].rearrange("p (w f) -> p w f",
                                                  f=PAIR_F)[:, :, HID:PAIR_F],
                            in0=vL[:, w0 * OUT:w1 * OUT].rearrange(
                                "p (w f) -> p w f", f=OUT),
                            in1=bc3(dinvb, w0, w1, OUT),
                            op=mybir.AluOpType.mult)
                        nc.sync.dma_start(
                            out=bnP[1][w0 * P:w1 * P, 0:PAIR_F]
                                .rearrange("(w p) f -> p w f", p=P),
                            in_=ublk[:].rearrange("p (w f) -> p w f", w=nw))
                    else:
                        # label-round-3 table rows (unmasked windows only)
                        if w1 > wU:
                            v0 = max(w0, wU)
                            nvb = w1 - v0
                            bblk = wk.tile([P, nvb * OUT], bf16, tag="bblk")
                            nc.vector.tensor_tensor(
                                out=bblk[:].rearrange("p (w f) -> p w f",
                                                      f=OUT),
                                in0=vL[:, v0 * OUT:w1 * OUT].rearrange(
                                    "p (w f) -> p w f", f=OUT),
                                in1=bc3(dinvb, v0, w1, OUT),
                                op=mybir.AluOpType.mult)
                            nc.sync.dma_start(
                                out=bnL[0][(v0 - wU) * P:(w1 - wU) * P, :]
                                    .rearrange("(w p) f -> p w f", p=P),
                                in_=bblk[:].rearrange("p (w f) -> p w f",
                                                      w=nvb))

                stage_agg(layP, tabP_aps(tab), idxP_d, ohP_d, PAIR_F, cb)

            paired_round(0)
            nc.gpsimd.collective_compute(
                "AllGather", bass.mybir.AluOpType.bypass,
                replica_groups=[list(range(NC))],
                ins=[bnP[1][:, :].opt()],
                outs=[tabP[1][0:TABROWS, :].opt()])
            paired_round(1)
            nc.gpsimd.collective_compute(
                "AllGather", bass.mybir.AluOpType.bypass,
                replica_groups=[list(range(NC))],
                ins=[bnL[0][:, :].opt()],
                outs=[tabL[0][0:NC * RU, :].opt()])

            # ======================================================================
            # 5. fuse fully-masked windows (xl = y, h final) under label rounds
            # ======================================================================
            for wb in range(0, wU, XB):
                nwb = min(XB, wU - wb)
                oblk = wk.tile([P, nwb * OUT], f32, tag="ofin")
                for a in range(nwb):
                    fuse_window(wb + a, oblk, a)
                nc.sync.dma_start(
                    out=out_sh[wb * P:(wb + nwb) * P, :]
                        .rearrange("(w p) f -> p w f", p=P),
                    in_=oblk[:].rearrange("p (w f) -> p w f", w=nwb))

            # ======================================================================
            # 6. label-only rounds 3..10
            # ======================================================================
            for j in range(3, NUM_LBL + 1):
                last = (j == NUM_LBL)
                ti = (j - 3) % 2

                def cb_lbl(s, w0, w1, pt, wtot, last=last, j=j, ti=ti):
                    nw = w1 - w0
                    Gblk = wk.tile([P, nw * OUT], f32, tag="Gblk")
                    G3 = Gblk[:].rearrange("p (w f) -> p w f", f=OUT)
                    vL3 = vL[:, w0 * OUT:w1 * OUT].rearrange(
                        "p (w f) -> p w f", f=OUT)
                    db = bc3(dinvb, w0, w1, OUT)
                    # G = dinv*(ps + c + dinv*vL)
                    nc.vector.tensor_tensor(out=G3, in0=vL3, in1=db,
                                            op=mybir.AluOpType.mult)
                    nc.vector.tensor_add(
                        out=Gblk[:], in0=Gblk[:],
                        in1=cbuf[:, (w0 - wU) * OUT:(w1 - wU) * OUT])
                    if pt is not None and all(wtot[w] > 0
                                              for w in range(w0, w1)):
                        nc.vector.tensor_add(out=Gblk[:], in0=Gblk[:],
                                             in1=pt[:, 0:nw * OUT])
                    elif pt is not None:
                        for w in range(w0, w1):
                            if wtot[w] > 0:
                                gsl = Gblk[:, (w - w0) * OUT:
                                           (w - w0 + 1) * OUT]
                                nc.vector.tensor_add(
                                    out=gsl, in0=gsl,
                                    in1=pt[:, (w - w0) * OUT:
                                           (w - w0 + 1) * OUT])
                    nc.vector.tensor_tensor(out=G3, in0=G3, in1=db,
                                            op=mybir.AluOpType.mult)

                    def dest(w, upsl):
                        nc.vector.tensor_copy(
                            out=vL[:, w * OUT:(w + 1) * OUT], in_=upsl)
                    apply_Wl(j - 1, Gblk, w0, nw, dest)
                    nc.vector.copy_predicated(
                        out=vL3, mask=bc3(maskb, w0, w1, OUT),
                        data=yb[:, w0 * OUT:w1 * OUT].rearrange(
                            "p (w f) -> p w f", f=OUT))

                    if not last:
                        bblk = wk.tile([P, nw * OUT], bf16, tag="bblk")
                        nc.vector.tensor_tensor(
                            out=bblk[:].rearrange("p (w f) -> p w f", f=OUT),
                            in0=vL3, in1=db, op=mybir.AluOpType.mult)
                        nc.sync.dma_start(
                            out=bnL[(ti + 1) % 2][(w0 - wU) * P:(w1 - wU) * P, :]
                                .rearrange("(w p) f -> p w f", p=P),
                            in_=bblk[:].rearrange("p (w f) -> p w f", w=nw))
                    else:
                        oblk = wk.tile([P, nw * OUT], f32, tag="ofin")
                        for a in range(nw):
                            fuse_window(w0 + a, oblk, a)
                        nc.sync.dma_start(
                            out=out_sh[w0 * P:w1 * P, :]
                                .rearrange("(w p) f -> p w f", p=P),
                            in_=oblk[:].rearrange("p (w f) -> p w f", w=nw))

                stage_agg(layL, tab4_aps(tabL[ti], NC * RU + 8), None, ohL_d,
                          OUT, cb_lbl)
                if not last:
                    nc.gpsimd.collective_compute(
                        "AllGather", bass.mybir.AluOpType.bypass,
                        replica_groups=[list(range(NC))],
                        ins=[bnL[(ti + 1) % 2][:, :].opt()],
                        outs=[tabL[(ti + 1) % 2][0:NC * RU, :].opt()])

    nc.compile()
    return nc


_CACHE = {}


def kernel(x, y, edge_index, deep_walk_emb, label_input_mask,
           W_gcn0, b_gcn0, W_gcn1, b_gcn1, W_label, b_label, W_fuse, b_fuse):
    import concourse.bass_utils as bass_utils
    import ml_dtypes

    n_nodes = x.shape[0]
    ei = np.asarray(edge_index, dtype=np.int64)
    meta = _preprocess(ei, np.asarray(label_input_mask), n_nodes)
    core_of, local_of = meta["core_of"], meta["local_of"]
    layP, layL, layC = meta["layP"], meta["layL"], meta["layC"]

    nonzero_b = (bool(np.any(np.asarray(b_gcn0))),
                 bool(np.any(np.asarray(b_gcn1))),
                 bool(np.any(np.asarray(b_label))),
                 bool(np.any(np.asarray(b_fuse))))
    if nonzero_b[2]:
        raise NotImplementedError("nonzero label bias not wired")

    key = ("v2", n_nodes, ei.shape[1], nonzero_b, meta["wU"],
           layP["totcols"], layL["totcols"], layC["totcols"],
           layP["ncols"].tobytes(), layL["ncols"].tobytes(),
           layC["ncols"].tobytes())
    if key not in _CACHE:
        _CACHE[key] = _build(meta, nonzero_b)
    nc = _CACHE[key]

    x_s = _shard_nodes(x, core_of, local_of, IN_DIM)
    y_s = _shard_nodes(y, core_of, local_of, OUT)
    dw_s = _shard_nodes(deep_walk_emb, core_of, local_of, DW)
    mk_s = _shard_nodes(np.asarray(label_input_mask, np.int8)[:, None],
                        core_of, local_of, 1, dtype=np.int8)
    dg_s = np.zeros((NC, NPAD, 1), np.int32)
    dg_s[core_of, local_of, 0] = meta["deg"].astype(np.int32)

    bmax = max(HID, OUT) * NUM_LBL
    b_all = np.zeros((4, bmax), np.float32)
    b_all[0, :HID] = np.asarray(b_gcn0, np.float32)
    b_all[1, :HID] = np.asarray(b_gcn1, np.float32)
    b_all[2, :OUT * NUM_LBL] = np.asarray(b_label, np.float32).reshape(-1)
    b_all[3, :OUT] = np.asarray(b_fuse, np.float32)

    Wl_flat = np.asarray(W_label, np.float32).reshape(NUM_LBL * OUT, OUT)
    idxP128 = np.tile(layP["idx16"], (1, 8, 1))
    idxL128 = np.tile(layL["idx16"], (1, 8, 1))
    idxC128 = np.tile(layC["idx16"], (1, 8, 1))

    def onehots(dpart):
        ncc, pp_, tcc = dpart.shape
        out = np.empty((ncc, pp_, tcc * 128), ml_dtypes.bfloat16)
        ar = np.arange(128, dtype=np.float32)
        for c in range(ncc):
            out[c] = (dpart[c][:, :, None] == ar).reshape(
                pp_, tcc * 128).astype(ml_dtypes.bfloat16)
        return out

    ohP = onehots(layP["dpart"])
    ohL = onehots(layL["dpart"])
    ohC = onehots(layC["dpart"])

    in_maps = []
    for c in range(NC):
        in_maps.append({
            "x_sh": x_s[c], "y_sh": y_s[c], "dw_sh": dw_s[c],
            "mask_sh": mk_s[c], "deg_sh": dg_s[c],
            "idxP_d": idxP128[c], "idxL_d": idxL128[c], "idxC_d": idxC128[c],
            "ohP_d": ohP[c], "ohL_d": ohL[c], "ohC_d": ohC[c],
            "W0": np.asarray(W_gcn0, np.float32),
            "W1": np.asarray(W_gcn1, np.float32),
            "Wl": Wl_flat,
            "Wf": np.asarray(W_fuse, np.float32),
            "b_all": b_all,
        })
    res = bass_utils.run_bass_kernel_spmd(nc, in_maps, core_ids=list(range(NC)))
    out = np.empty((n_nodes, OUT), np.float32)
    for c in range(NC):
        sel = core_of == np.int64(c)
        out[sel] = res.results[c]["out_sh"][local_of[sel]]
    return out


# revision 23
# speedup vs baseline: 1.0289x; 1.0289x over previous
"""FPLPGCN (2x GCNConv feature prop + 10x label prop + fuse) on 8 trn2 cores.

Strategy (graph/data parallel, v2):
- Nodes sorted by (mask, in-degree), striped round-robin across 8 cores.
  Masked (label-overwritten) nodes occupy low local rows; unmasked high rows
  [B*, NPAD).  Each core owns NPAD=12544 rows (44 fake pad rows).
- GCN refactor: out[n] = dinv[n]*(sum_{e->n} u'[src] + u'[n]) + b with
  u' = dinv*(z @ W).  Self terms are computed locally (no self edges).
- Rounds r0/r1 use a paired table [u_feat 64 | t_lbl 32 | pad 32] bf16 so one
  gather serves both chains.  The label chain postpones its 32x32 weight to
  AFTER aggregation, so the label table holds t = dinv*xl directly.
- Label-only rounds 3..10 gather ONLY unmasked-src -> unmasked-dst edges.
  Masked sources always contribute c[n] = sum dinv*y[src] (constant): c is
  aggregated once at startup from a dinv*y table (layC).
- Aggregation: dense-packed dma_gather slots (256B elems); per 128-slot
  column a one-hot (dst-partition) matrix streamed from DRAM routes messages
  into per-window PSUM via PE matmul-accumulation.  Post-processing
  (self terms, weights, mask overwrite, bounce prep) is batched into
  per-span 3D-broadcast vector ops.
"""

import sys

sys.path.insert(0, "/opt/trn_rl_repo")

import numpy as np

NC = 8
P = 128
NPAD = 12544           # local rows per core (98 windows of 128)
NWIN = NPAD // P       # 98
TABROWS = NC * NPAD    # 100352
CHP = TABROWS // 4     # 25088: paired chunk (row stride 256B)
IN_DIM, HID, OUT, DW = 128, 64, 32, 64
PAIR_F = HID + OUT     # 96 useful cols of the 128-wide paired row
NUM_LBL = 10
SPAN_P = 5             # windows per paired span (psum: 5*96*4B = 1920B/bank)
SPAN_L = 10            # windows per label span (psum: 10*32*4B = 1280B)
MAXCOLS = 24           # <=3072 idxs per dma_gather call


def _cdiv(a, b):
    return -(-a // b)


# ----------------------------------------------------------------------------
# host-side index preprocessing (pure index manipulation; no FP math on data)
# ----------------------------------------------------------------------------

def _build_layout(ecore, ewin, epart, grp, idxv, spans):
    """Dense slot layout for one gather addressing scheme."""
    EA = ecore.shape[0]
    key = (ecore * NWIN + ewin) * 4 + grp
    cnt = np.bincount(key, minlength=NC * NWIN * 4).reshape(NC, NWIN, 4)
    ncols = _cdiv(cnt.max(axis=0), 128)                     # [NWIN, 4]

    col_start = np.zeros((NWIN, 4), np.int64)
    region_col0 = np.zeros((len(spans), 4), np.int64)
    region_off16 = np.zeros((len(spans), 4), np.int64)
    col = 0
    off16 = 0
    span_of_w = np.full(NWIN, -1, np.int64)
    for s, (w0, w1) in enumerate(spans):
        span_of_w[w0:w1] = s
        for g in range(4):
            region_col0[s, g] = col
            region_off16[s, g] = off16
            for w in range(w0, w1):
                col_start[w, g] = col
                col += int(ncols[w, g])
            off16 += int((col - region_col0[s, g]) * 8)
    totcols = col
    tot16 = off16

    o = np.argsort(key, kind="stable")
    ks = key[o]
    first = np.searchsorted(ks, ks, side="left")
    pos = np.empty(EA, np.int64)
    pos[o] = np.arange(EA) - first

    ecolg = col_start[ewin, grp] + pos // 128
    part = pos % 128
    es = span_of_w[ewin]
    i_in_region = (ecolg - region_col0[es, grp]) * 128 + part

    idx16 = np.zeros((NC, 16, max(tot16, 8)), np.int16)
    idx16[ecore, i_in_region % 16,
          region_off16[es, grp] + i_in_region // 16] = idxv.astype(np.int16)

    dpart = np.full((NC, 128, max(totcols, 1)), 128.0, np.float32)
    dpart[ecore, part, ecolg] = epart.astype(np.float32)

    return dict(idx16=idx16, dpart=dpart, ncols=ncols, spans=spans,
                region_col0=region_col0, region_off16=region_off16,
                totcols=totcols, tot16=tot16)


def _preprocess(edge_index, mask, n_nodes):
    src = np.ascontiguousarray(edge_index[0]).astype(np.int64)
    dst = np.ascontiguousarray(edge_index[1]).astype(np.int64)
    deg = np.bincount(dst, minlength=n_nodes).astype(np.int64)
    mask = np.ascontiguousarray(mask).astype(bool)
    nfake = TABROWS - n_nodes

    # masked nodes first, then unmasked; degree-sorted within each group
    order = np.lexsort((deg, (~mask).astype(np.int64)))
    rank = np.empty(n_nodes, np.int64)
    rank[order] = np.arange(nfake, TABROWS)
    core_of = rank % NC
    local_of = rank // NC
    trow = core_of * NPAD + local_of

    M = int(mask.sum())
    wU = (nfake + M) // (NC * P)          # first label-updated window
    assert 1 <= wU <= NWIN - 1
    BST = wU * P                          # boundary local row (window aligned)
    RU = NPAD - BST                       # label-table rows per core

    spansP = [(w0, min(w0 + SPAN_P, NWIN)) for w0 in range(0, NWIN, SPAN_P)]
    spansL = [(w0, min(w0 + SPAN_L, NWIN)) for w0 in range(wU, NWIN, SPAN_L)]

    layP = _build_layout(core_of[dst], local_of[dst] // P, local_of[dst] % P,
                         trow[src] // CHP, trow[src] % CHP, spansP)

    selL = (local_of[src] >= BST) & (local_of[dst] >= BST)
    tL = core_of * RU + (local_of - BST)
    sL, dL = src[selL], dst[selL]
    layL = _build_layout(core_of[dL], local_of[dL] // P, local_of[dL] % P,
                         tL[sL] % 4, tL[sL] // 4, spansL)

    selC = (local_of[src] < BST) & (local_of[dst] >= BST)
    tC = core_of * BST + local_of
    sC, dC = src[selC], dst[selC]
    layC = _build_layout(core_of[dC], local_of[dC] // P, local_of[dC] % P,
                         tC[sC] % 4, tC[sC] // 4, spansL)

    meta = dict(order=order, core_of=core_of, local_of=local_of,
                deg=deg, wU=wU, BST=BST, RU=RU,
                layP=layP, layL=layL, layC=layC)
    return meta


def _shard_nodes(arr, core_of, local_of, width, dtype=np.float32):
    n = arr.shape[0]
    out = np.zeros((NC, NPAD, width), dtype)
    a2 = np.asarray(arr, dtype).reshape(n, width)
    out[core_of, local_of] = a2
    return out


# ----------------------------------------------------------------------------
# device program
# ----------------------------------------------------------------------------

def _build(meta, nonzero_b):
    import concourse.bacc as bacc
    import concourse.bass as bass
    import concourse.mybir as mybir
    import concourse.tile as tile
    from concourse.bass import AP

    f32 = mybir.dt.float32
    bf16 = mybir.dt.bfloat16
    layP, layL, layC = meta["layP"], meta["layL"], meta["layC"]
    wU, BST, RU = meta["wU"], meta["BST"], meta["RU"]
    NWU = NWIN - wU

    nc = bacc.Bacc("TRN2", target_bir_lowering=False, debug=False,
                   num_devices=NC, num_swdge_queues=4)

    x_sh = nc.dram_tensor("x_sh", [NPAD, IN_DIM], f32, kind="ExternalInput")
    y_sh = nc.dram_tensor("y_sh", [NPAD, OUT], f32, kind="ExternalInput")
    dw_sh = nc.dram_tensor("dw_sh", [NPAD, DW], f32, kind="ExternalInput")
    mask_sh = nc.dram_tensor("mask_sh", [NPAD, 1], mybir.dt.int8,
                             kind="ExternalInput")
    deg_sh = nc.dram_tensor("deg_sh", [NPAD, 1], mybir.dt.int32,
                            kind="ExternalInput")
    idxP_d = nc.dram_tensor("idxP_d", [P, layP["tot16"]], mybir.dt.int16,
                            kind="ExternalInput")
    idxL_d = nc.dram_tensor("idxL_d", [P, layL["tot16"]], mybir.dt.int16,
                            kind="ExternalInput")
    idxC_d = nc.dram_tensor("idxC_d", [P, layC["tot16"]], mybir.dt.int16,
                            kind="ExternalInput")
    ohP_d = nc.dram_tensor("ohP_d", [P, layP["totcols"] * P], bf16,
                           kind="ExternalInput")
    ohL_d = nc.dram_tensor("ohL_d", [P, layL["totcols"] * P], bf16,
                           kind="ExternalInput")
    ohC_d = nc.dram_tensor("ohC_d", [P, layC["totcols"] * P], bf16,
                           kind="ExternalInput")
    W0_d = nc.dram_tensor("W0", [IN_DIM, HID], f32, kind="ExternalInput")
    W1_d = nc.dram_tensor("W1", [HID, HID], f32, kind="ExternalInput")
    Wl_d = nc.dram_tensor("Wl", [NUM_LBL * OUT, OUT], f32, kind="ExternalInput")
    Wf_d = nc.dram_tensor("Wf", [HID + OUT + DW, OUT], f32, kind="ExternalInput")
    b_d = nc.dram_tensor("b_all", [4, max(HID, OUT) * NUM_LBL], f32,
                         kind="ExternalInput")
    out_sh = nc.dram_tensor("out_sh", [NPAD, OUT], f32, kind="ExternalOutput")

    # internal DRAM
    tabP = [nc.dram_tensor(f"tabP{i}", [TABROWS, P], bf16,
                           addr_space="Shared") for i in range(2)]
    tabL = [nc.dram_tensor(f"tabL{i}", [NC * RU + 8, OUT], bf16,
                           addr_space="Shared") for i in range(2)]
    tabC = nc.dram_tensor("tabC", [NC * BST + 8, OUT], bf16,
                          addr_space="Shared")
    bnP = [nc.dram_tensor(f"bnP{i}", [NPAD, P], bf16) for i in range(2)]
    bnL = [nc.dram_tensor(f"bnL{i}", [RU, OUT], bf16) for i in range(2)]
    bnC = nc.dram_tensor("bnC", [BST, OUT], bf16)

    with tile.TileContext(nc) as tc:
        with tc.tile_pool(name="persist", bufs=1) as pp, \
             tc.tile_pool(name="g", bufs=7) as gp, \
             tc.tile_pool(name="ix", bufs=3) as ixp, \
             tc.tile_pool(name="oh", bufs=3) as ohp, \
             tc.tile_pool(name="wk", bufs=3) as wk, \
             tc.tile_pool(name="ps", bufs=2, space="PSUM") as ps, \
             tc.tile_pool(name="psu", bufs=2, space="PSUM") as psu, \
             tc.tile_pool(name="psw", bufs=4, space="PSUM") as psw:

            # ---- constants / persistent state ----
            W0 = pp.tile([IN_DIM, HID], f32)
            nc.sync.dma_start(out=W0[:], in_=W0_d[:, :])
            W1r = pp.tile([P, HID], f32)
            for a in range(2):
                nc.sync.dma_start(out=W1r[a * HID:(a + 1) * HID, :],
                                  in_=W1_d[:, :])
            Wl4 = pp.tile([P, NUM_LBL * OUT], f32)
            for a in range(3):
                nc.sync.dma_start(
                    out=Wl4[a * OUT:(a + 1) * OUT, :]
                        .rearrange("p (j f) -> p j f", j=NUM_LBL),
                    in_=Wl_d[:, :].rearrange("(j k) f -> k j f", k=OUT))
            Wfa = pp.tile([128, OUT], f32)
            nc.sync.dma_start(out=Wfa[:], in_=Wf_d[0:128, :])
            Wfb = pp.tile([HID + OUT + DW - 128, OUT], f32)
            nc.sync.dma_start(out=Wfb[:], in_=Wf_d[128:, :])
            from concourse.masks import make_identity
            ident = pp.tile([P, P], f32)
            make_identity(nc, ident[:])
            onecol = pp.tile([1, P], f32)
            nc.vector.memset(onecol[:], 1.0)

            yb = pp.tile([P, NWIN * OUT], f32)
            nc.sync.dma_start(
                out=yb[:].rearrange("p (w f) -> p w f", w=NWIN),
                in_=y_sh[:, :].rearrange("(w p) f -> p w f", p=P))
            maskb = pp.tile([P, NWIN], mybir.dt.int8)
            nc.sync.dma_start(
                out=maskb[:], in_=mask_sh[:, 0].rearrange("(w p) -> p w", p=P))
            degb = pp.tile([P, NWIN], mybir.dt.int32)
            nc.sync.dma_start(
                out=degb[:], in_=deg_sh[:, 0].rearrange("(w p) -> p w", p=P))

            degf = pp.tile([P, NWIN], f32)
            nc.vector.tensor_copy(out=degf[:], in_=degb[:])
            recipb = pp.tile([P, NWIN], f32)
            nc.vector.tensor_scalar(out=degf[:], in0=degf[:], scalar1=1.0,
                                    scalar2=None, op0=mybir.AluOpType.add)
            nc.vector.reciprocal(out=recipb[:], in_=degf[:])      # 1/(deg+1)
            dinvb = pp.tile([P, NWIN], f32)
            nc.scalar.sqrt(out=dinvb[:], in_=recipb[:])           # 1/sqrt(deg+1)
            nfake = TABROWS - 100000
            nc.vector.memset(recipb[0:nfake // NC, 0:1], 0.0)
            nc.vector.memset(dinvb[0:nfake // NC, 0:1], 0.0)
            dinvy = pp.tile([P, NWIN * OUT], f32)
            for w in range(NWIN):
                nc.vector.tensor_scalar(
                    out=dinvy[:, w * OUT:(w + 1) * OUT],
                    in0=yb[:, w * OUT:(w + 1) * OUT],
                    scalar1=dinvb[:, w:w + 1], scalar2=None,
                    op0=mybir.AluOpType.mult)

            def bias_tile(row, width):
                bt = pp.tile([P, width], f32, tag=f"bias{row}", name=f"bias{row}")
                brow = pp.tile([1, width], f32, tag=f"brow{row}", name=f"brow{row}")
                nc.sync.dma_start(out=brow[:], in_=b_d[row:row + 1, 0:width])
                pt = ps.tile([P, P], f32, tag="tps", name="biasps")
                nc.tensor.matmul(out=pt[:, 0:width], lhsT=onecol[:],
                                 rhs=brow[:], start=True, stop=True)
                nc.vector.tensor_copy(out=bt[:], in_=pt[:, 0:width])
                return bt

            bias0 = bias_tile(0, HID) if nonzero_b[0] else None
            bias1 = bias_tile(1, HID) if nonzero_b[1] else None
            biasf = bias_tile(3, OUT) if nonzero_b[3] else None

            vF = pp.tile([P, NWIN * HID], f32)      # feature state (h at end)
            vL = pp.tile([P, NWIN * OUT], f32)      # label state xl (plain)
            ulocF = pp.tile([P, NWIN * HID], bf16)  # own feature table rows
            cbuf = pp.tile([P, NWU * OUT], f32)     # masked-src label constant

            # resident index table for the 8 label rounds
            ixL = pp.tile([P, layL["tot16"]], mybir.dt.int16)
            nc.sync.dma_start(out=ixL[:], in_=idxL_d[:, :])

            # vL starts as y (masked windows keep it forever)
            nc.vector.tensor_copy(out=vL[:], in_=yb[:])

            qctr = [0]

            def bc3(t, w0, w1, F):
                """[P, w1-w0, F] view of t[:, w0:w1] with 0-stride inner."""
                sl = t[:, w0:w1]
                return AP(sl.tensor, sl.offset,
                          [sl.ap[0], sl.ap[1], [0, F]])

            # ---- aggregation engine ----
            def stage_agg(lay, tab_in_aps, ix_src, oh_src, F, span_cb):
                """Gather + one-hot matmul segment-sum.

                ix_src: None -> resident (use ix_tile slices); else DRAM tensor.
                span_cb(s, w0, w1, pt, wtot): called once per span after its
                accumulation chain closes; pt[:, (w-w0)*F:...] holds window w's
                aggregate (only valid when wtot[w] > 0).
                """
                ncols = lay["ncols"]
                spans = lay["spans"]
                region_col0 = lay["region_col0"]
                region_off16 = lay["region_off16"]
                for s, (w0, w1) in enumerate(spans):
                    span_off16 = int(region_off16[s, 0])
                    span_cols = int(ncols[w0:w1, :].sum())
                    wtot = {w: int(ncols[w, :].sum()) for w in range(w0, w1)}
                    nw = w1 - w0
                    if span_cols == 0:
                        span_cb(s, w0, w1, None, wtot)
                        continue
                    span_n16 = span_cols * 8
                    if ix_src is None:
                        ixt = ixL
                        ix_base = span_off16
                    else:
                        ixt = ixp.tile([P, span_n16], mybir.dt.int16, tag="ix")
                        nc.sync.dma_start(
                            out=ixt[:],
                            in_=ix_src[:, span_off16:span_off16 + span_n16])
                        ix_base = 0

                    pt = psw.tile([P, nw * F], f32, tag="aggps", name="aggps")
                    kspan = [0]
                    kspan_tot = int(sum(wtot.values()))

                    for g in range(4):
                        rcols = int(ncols[w0:w1, g].sum())
                        if rcols == 0:
                            continue
                        o16 = ix_base + int(region_off16[s, g]) - span_off16
                        rcol0 = int(region_col0[s, g])
                        tiles = []
                        ohts = []
                        for c0 in range(0, rcols, MAXCOLS):
                            c1 = min(c0 + MAXCOLS, rcols)
                            nidx = (c1 - c0) * P
                            gt = gp.tile([P, (c1 - c0) * P], bf16, tag="g")
                            nc.gpsimd.dma_gather(
                                out_ap=gt[:].rearrange("p (s f) -> p s f", f=P),
                                in_ap=tab_in_aps[g],
                                idxs_ap=ixt[:, o16 + c0 * 8:o16 + c1 * 8],
                                num_idxs=nidx, num_idxs_reg=nidx,
                                elem_size=P, queue_num=qctr[0] % 4,
                                single_packet=False)
                            qctr[0] += 1
                            tiles.append(gt)
                            oht = ohp.tile([P, (c1 - c0) * P], bf16, tag="oh")
                            nc.sync.dma_start(
                                out=oht[:],
                                in_=oh_src[:, (rcol0 + c0) * P:(rcol0 + c1) * P])
                            ohts.append(oht)
                        creg = 0
                        for w in range(w0, w1):
                            n = int(ncols[w, g])
                            pslice = pt[:, (w - w0) * F:(w - w0 + 1) * F]
                            for c in range(n):
                                cr = creg + c
                                gt = tiles[cr // MAXCOLS]
                                oht = ohts[cr // MAXCOLS]
                                toff = cr % MAXCOLS
                                ks = kspan[0]
                                nc.tensor.matmul(
                                    out=pslice,
                                    lhsT=oht[:, toff * P:(toff + 1) * P],
                                    rhs=gt[:, toff * P:toff * P + F],
                                    start=(ks == 0),
                                    stop=(ks == kspan_tot - 1))
                                kspan[0] = ks + 1
                            creg += n
                    span_cb(s, w0, w1, pt, wtot)

            def tabP_aps(t):
                return [t[q * CHP:(q + 1) * CHP, :] for q in range(4)]

            def tab4_aps(t, nrows_alloc):
                k4 = (nrows_alloc - 4) // 4
                return [t[q:q + 4 * k4, :].rearrange(
                    "(r k) f -> r (k f)", k=4) for q in range(4)]

            # ---- shared post-processing helpers ----
            def apply_Wl(j, Gblk, w0, nw, dest_cb):
                """dest_cb(w, up_slice_ap) with up = Gblk@W_label[j] per window."""
                for b0 in range(0, nw, 3):
                    b1 = min(b0 + 3, nw)
                    nb = b1 - b0
                    tp = ps.tile([P, P], f32, tag="tps")
                    nc.tensor.transpose(
                        out=tp[0:nb * OUT, :],
                        in_=Gblk[:, b0 * OUT:b1 * OUT], identity=ident[:])
                    vT = wk.tile([P, P], f32, tag="vT")
                    nc.scalar.copy(out=vT[0:nb * OUT, :], in_=tp[0:nb * OUT, :])
                    for a in range(nb):
                        up = psu.tile([P, OUT], f32, tag="ups")
                        nc.tensor.matmul(
                            out=up[:],
                            lhsT=vT[a * OUT:(a + 1) * OUT, :],
                            rhs=Wl4[a * OUT:(a + 1) * OUT,
                                    j * OUT:(j + 1) * OUT],
                            start=True, stop=True)
                        dest_cb(w0 + b0 + a, up[:])

            def fuse_window(w, oblk, ob_off):
                dwt = wk.tile([P, DW], f32, tag="dwt")
                nc.sync.dma_start(out=dwt[:], in_=dw_sh[w * P:(w + 1) * P, :])
                fTa = wk.tile([P, P], f32, tag="fTa")
                fTb = wk.tile([DW - 32, P], f32, tag="fTb")
                tp = ps.tile([P, P], f32, tag="tps")
                nc.tensor.transpose(out=tp[0:HID, :],
                                    in_=vF[:, w * HID:(w + 1) * HID],
                                    identity=ident[:])
                nc.scalar.copy(out=fTa[0:HID, :], in_=tp[0:HID, :])
                tp2 = ps.tile([P, P], f32, tag="tps")
                nc.tensor.transpose(out=tp2[0:OUT, :],
                                    in_=vL[:, w * OUT:(w + 1) * OUT],
                                    identity=ident[:])
                nc.scalar.copy(out=fTa[HID:HID + OUT, :], in_=tp2[0:OUT, :])
                tp3 = ps.tile([P, P], f32, tag="tps")
                nc.tensor.transpose(out=tp3[0:DW, :], in_=dwt[:],
                                    identity=ident[:])
                nc.scalar.copy(out=fTa[HID + OUT:P, :],
                               in_=tp3[0:P - HID - OUT, :])
                nc.scalar.copy(out=fTb[:, :], in_=tp3[P - HID - OUT:DW, :])
                op = psu.tile([P, OUT], f32, tag="ups", name="ops")
                nc.tensor.matmul(out=op[:], lhsT=fTa[:], rhs=Wfa[:],
                                 start=True, stop=False)
                nc.tensor.matmul(out=op[:], lhsT=fTb[:], rhs=Wfb[:],
                                 start=False, stop=True)
                if biasf is not None:
                    nc.vector.tensor_add(out=op[:], in0=op[:], in1=biasf[:])
                nc.scalar.activation(
                    out=oblk[:, ob_off * OUT:(ob_off + 1) * OUT], in_=op[:],
                    func=bass.mybir.ActivationFunctionType.Sigmoid)

            # ======================================================================
            # 1. bnC = bf16(dinvy rows [0, BST)) ; AG_C -> tabC
            # ======================================================================
            CB = 7
            for wb in range(0, wU, CB):
                nwb = min(CB, wU - wb)
                cblk = wk.tile([P, nwb * OUT], bf16, tag="cblk")
                nc.vector.tensor_copy(
                    out=cblk[:], in_=dinvy[:, wb * OUT:(wb + nwb) * OUT])
                nc.sync.dma_start(
                    out=bnC[wb * P:(wb + nwb) * P, :]
                        .rearrange("(w p) f -> p w f", p=P),
                    in_=cblk[:].rearrange("p (w f) -> p w f", w=nwb))
            nc.gpsimd.collective_compute(
                "AllGather", bass.mybir.AluOpType.bypass,
                replica_groups=[list(range(NC))],
                ins=[bnC[:, :].opt()],
                outs=[tabC[0:NC * BST, :].opt()])

            # ======================================================================
            # 2. init tables: [dinv*(x@W0) | dinvy] -> bnP[0] ; AG0
            # ======================================================================
            XB = 4
            for wb in range(0, NWIN, XB):
                nwb = min(XB, NWIN - wb)
                ublk = wk.tile([P, nwb * PAIR_F], bf16, tag="ubx")
                xt4 = wk.tile([P, nwb * IN_DIM], f32, tag="xt")
                nc.sync.dma_start(
                    out=xt4[:].rearrange("p (w f) -> p w f", w=nwb),
                    in_=x_sh[wb * P:(wb + nwb) * P, :]
                        .rearrange("(w p) f -> p w f", p=P))
                nc.vector.tensor_tensor(
                    out=xt4[:].rearrange("p (w f) -> p w f", f=IN_DIM),
                    in0=xt4[:].rearrange("p (w f) -> p w f", f=IN_DIM),
                    in1=bc3(dinvb, wb, wb + nwb, IN_DIM),
                    op=mybir.AluOpType.mult)
                for a in range(nwb):
                    w = wb + a
                    xt = xt4[:, a * IN_DIM:(a + 1) * IN_DIM]
                    tp = ps.tile([P, P], f32, tag="tps")
                    nc.tensor.transpose(out=tp[:], in_=xt, identity=ident[:])
                    vT = wk.tile([P, P], f32, tag="vT")
                    nc.scalar.copy(out=vT[:], in_=tp[:])
                    up = psu.tile([P, HID], f32, tag="ups")
                    nc.tensor.matmul(out=up[:], lhsT=vT[:], rhs=W0[:],
                                     start=True, stop=True)
                    nc.scalar.copy(out=ublk[:, a * PAIR_F:a * PAIR_F + HID],
                                   in_=up[:])
                    nc.scalar.copy(out=ulocF[:, w * HID:(w + 1) * HID],
                                   in_=up[:])
                    nc.vector.tensor_copy(
                        out=ublk[:, a * PAIR_F + HID:(a + 1) * PAIR_F],
                        in_=dinvy[:, w * OUT:(w + 1) * OUT])
                nc.sync.dma_start(
                    out=bnP[0][wb * P:(wb + nwb) * P, 0:PAIR_F]
                        .rearrange("(w p) f -> p w f", p=P),
                    in_=ublk[:].rearrange("p (w f) -> p w f", w=nwb))
            nc.gpsimd.collective_compute(
                "AllGather", bass.mybir.AluOpType.bypass,
                replica_groups=[list(range(NC))],
                ins=[bnP[0][:, :].opt()],
                outs=[tabP[0][0:TABROWS, :].opt()])

            # ======================================================================
            # 3. c aggregation (overlaps r0): cbuf = agg of dinv*y from masked srcs
            # ======================================================================
            def cb_c(s, w0, w1, pt, wtot):
                if pt is not None and all(wtot[w] > 0 for w in range(w0, w1)):
                    nc.vector.tensor_copy(
                        out=cbuf[:, (w0 - wU) * OUT:(w1 - wU) * OUT],
                        in_=pt[:, 0:(w1 - w0) * OUT])
                    return
                for w in range(w0, w1):
                    dst = cbuf[:, (w - wU) * OUT:(w - wU + 1) * OUT]
                    if pt is not None and wtot[w] > 0:
                        nc.vector.tensor_copy(
                            out=dst, in_=pt[:, (w - w0) * OUT:(w - w0 + 1) * OUT])
                    else:
                        nc.vector.memset(dst, 0.0)
            stage_agg(layC, tab4_aps(tabC, NC * BST + 8), idxC_d, ohC_d, OUT,
                      cb_c)

            # ======================================================================
            # 4. paired rounds r0 / r1
            # ======================================================================
            def paired_round(r):
                tab = tabP[r]
                jW = r                       # label weight index for this round

                def cb(s, w0, w1, pt, wtot):
                    nw = w1 - w0
                    Gblk = wk.tile([P, nw * OUT], f32, tag="Gblk")
                    pt3 = pt[:, 0:nw * PAIR_F].rearrange(
                        "p (w f) -> p w f", f=PAIR_F)
                    vF3 = vF[:, w0 * HID:w1 * HID].rearrange(
                        "p (w f) -> p w f", f=HID)
                    nc.vector.tensor_add(
                        out=vF3, in0=pt3[:, :, 0:HID],
                        in1=ulocF[:, w0 * HID:w1 * HID].rearrange(
                            "p (w f) -> p w f", f=HID))
                    sc = recipb if r == 0 else dinvb
                    nc.vector.tensor_tensor(out=vF3, in0=vF3,
                                            in1=bc3(sc, w0, w1, HID),
                                            op=mybir.AluOpType.mult)
                    if r == 0 and bias0 is not None:
                        for w in range(w0, w1):
                            dstF = vF[:, w * HID:(w + 1) * HID]
                            dv = wk.tile([P, HID], f32, tag="dbv")
                            nc.vector.tensor_scalar(out=dv[:], in0=bias0[:],
                                                    scalar1=dinvb[:, w:w + 1],
                                                    scalar2=None,
                                                    op0=mybir.AluOpType.mult)
                            nc.vector.tensor_add(out=dstF, in0=dstF, in1=dv[:])
                    if r == 1 and bias1 is not None:
                        for w in range(w0, w1):
                            dstF = vF[:, w * HID:(w + 1) * HID]
                            nc.vector.tensor_add(out=dstF, in0=dstF,
                                                 in1=bias1[:])
                    # label: G = dinv*(psL + self)
                    G3 = Gblk[:].rearrange("p (w f) -> p w f", f=OUT)
                    vL3 = vL[:, w0 * OUT:w1 * OUT].rearrange(
                        "p (w f) -> p w f", f=OUT)
                    db = bc3(dinvb, w0, w1, OUT)
                    if r == 0:
                        nc.vector.tensor_add(
                            out=G3, in0=pt3[:, :, HID:PAIR_F],
                            in1=dinvy[:, w0 * OUT:w1 * OUT].rearrange(
                                "p (w f) -> p w f", f=OUT))
                    else:
                        nc.vector.tensor_tensor(out=G3, in0=vL3, in1=db,
                                                op=mybir.AluOpType.mult)
                        nc.vector.tensor_add(out=G3, in0=G3,
                                             in1=pt3[:, :, HID:PAIR_F])
                    nc.vector.tensor_tensor(out=G3, in0=G3, in1=db,
                                            op=mybir.AluOpType.mult)

                    def dest(w, upsl):
                        nc.vector.tensor_copy(
                            out=vL[:, w * OUT:(w + 1) * OUT], in_=upsl)
                    apply_Wl(jW, Gblk, w0, nw, dest)
                    nc.vector.copy_predicated(
                        out=vL3, mask=bc3(maskb, w0, w1, OUT),
                        data=yb[:, w0 * OUT:w1 * OUT].rearrange(
                            "p (w f) -> p w f", f=OUT))

                    if r == 0:
                        # build r1 paired table rows for these windows
                        ublk = wk.tile([P, nw * PAIR_F], bf16, tag="ubp")
                        for b0 in range(0, nw, 2):
                            b1 = min(b0 + 2, nw)
                            nb = b1 - b0
                            tp = ps.tile([P, P], f32, tag="tps")
                            nc.tensor.transpose(
                                out=tp[0:nb * HID, :],
                                in_=vF[:, (w0 + b0) * HID:(w0 + b1) * HID],
                                identity=ident[:])
                            vT = wk.tile([P, P], f32, tag="vT")
                            nc.scalar.copy(out=vT[0:nb * HID, :],
                                           in_=tp[0:nb * HID, :])
                            for a in range(nb):
                                w = w0 + b0 + a
                                up = psu.tile([P, HID], f32, tag="ups")
                                nc.tensor.matmul(
                                    out=up[:],
                                    lhsT=vT[a * HID:(a + 1) * HID, :],
                                    rhs=W1r[a * HID:(a + 1) * HID, :],
                                    start=True, stop=True)
                                nc.scalar.copy(
                                    out=ublk[:, (b0 + a) * PAIR_F:
                                             (b0 + a) * PAIR_F + HID],
                                    in_=up[:])
                                nc.scalar.copy(
                                    out=ulocF[:, w * HID:(w + 1) * HID],
                                    in_=up[:])
                        nc.vector.tensor_tensor(
                            out=ublk[:].rearrange("p (w f) -> p w f",
                                                  f=PAIR_F)[:, :, HID:PAIR_F],
                            in0=vL[:, w0 * OUT:w1 * OUT].rearrange(
                                "p (w f) -> p w f", f=OUT),
                            in1=bc3(dinvb, w0, w1, OUT),
                            op=mybir.AluOpType.mult)
                        nc.sync.dma_start(
                            out=bnP[1][w0 * P:w1 * P, 0:PAIR_F]
                                .rearrange("(w p) f -> p w f", p=P),
                            in_=ublk[:].rearrange("p (w f) -> p w f", w=nw))
                    else:
                        # label-round-3 table rows (unmasked windows only)
                        if w1 > wU:
                            v0 = max(w0, wU)
                            nvb = w1 - v0
                            bblk = wk.tile([P, nvb * OUT], bf16, tag="bblk")
                            nc.vector.tensor_tensor(
                                out=bblk[:].rearrange("p (w f) -> p w f",
                                                      f=OUT),
                                in0=vL[:, v0 * OUT:w1 * OUT].rearrange(
                                    "p (w f) -> p w f", f=OUT),
                                in1=bc3(dinvb, v0, w1, OUT),
                                op=mybir.AluOpType.mult)
                            nc.sync.dma_start(
                                out=bnL[0][(v0 - wU) * P:(w1 - wU) * P, :]
                                    .rearrange("(w p) f -> p w f", p=P),
                                in_=bblk[:].rearrange("p (w f) -> p w f",
                                                      w=nvb))

                stage_agg(layP, tabP_aps(tab), idxP_d, ohP_d, PAIR_F, cb)

            paired_round(0)
            nc.gpsimd.collective_compute(
                "AllGather", bass.mybir.AluOpType.bypass,
                replica_groups=[list(range(NC))],
                ins=[bnP[1][:, :].opt()],
                outs=[tabP[1][0:TABROWS, :].opt()])
            paired_round(1)
            nc.gpsimd.collective_compute(
                "AllGather", bass.mybir.AluOpType.bypass,
                replica_groups=[list(range(NC))],
                ins=[bnL[0][:, :].opt()],
                outs=[tabL[0][0:NC * RU, :].opt()])

            # ======================================================================
            # 5. fuse fully-masked windows (xl = y, h final) under label rounds
            # ======================================================================
            for wb in range(0, wU, XB):
                nwb = min(XB, wU - wb)
                oblk = wk.tile([P, nwb * OUT], f32, tag="ofin")
                for a in range(nwb):
                    fuse_window(wb + a, oblk, a)
                nc.sync.dma_start(
                    out=out_sh[wb * P:(wb + nwb) * P, :]
                        .rearrange("(w p) f -> p w f", p=P),
                    in_=oblk[:].rearrange("p (w f) -> p w f", w=nwb))

            # ======================================================================
            # 6. label-only rounds 3..10
            # ======================================================================
            for j in range(3, NUM_LBL + 1):
                last = (j == NUM_LBL)
                ti = (j - 3) % 2

                def cb_lbl(s, w0, w1, pt, wtot, last=last, j=j, ti=ti):
                    nw = w1 - w0
                    Gblk = wk.tile([P, nw * OUT], f32, tag="Gblk")
                    G3 = Gblk[:].rearrange("p (w f) -> p w f", f=OUT)
                    vL3 = vL[:, w0 * OUT:w1 * OUT].rearrange(
                        "p (w f) -> p w f", f=OUT)
                    db = bc3(dinvb, w0, w1, OUT)
                    # G = dinv*(ps + c + dinv*vL)
                    nc.vector.tensor_tensor(out=G3, in0=vL3, in1=db,
                                            op=mybir.AluOpType.mult)
                    nc.vector.tensor_add(
                        out=Gblk[:], in0=Gblk[:],
                        in1=cbuf[:, (w0 - wU) * OUT:(w1 - wU) * OUT])
                    if pt is not None and all(wtot[w] > 0
                                              for w in range(w0, w1)):
                        nc.vector.tensor_add(out=Gblk[:], in0=Gblk[:],
                                             in1=pt[:, 0:nw * OUT])
                    elif pt is not None:
                        for w in range(w0, w1):
                            if wtot[w] > 0:
                                gsl = Gblk[:, (w - w0) * OUT:
                                           (w - w0 + 1) * OUT]
                                nc.vector.tensor_add(
                                    out=gsl, in0=gsl,
                                    in1=pt[:, (w - w0) * OUT:
                                           (w - w0 + 1) * OUT])
                    nc.vector.tensor_tensor(out=G3, in0=G3, in1=db,
                                            op=mybir.AluOpType.mult)

                    def dest(w, upsl):
                        nc.vector.tensor_copy(
                            out=vL[:, w * OUT:(w + 1) * OUT], in_=upsl)
                    apply_Wl(j - 1, Gblk, w0, nw, dest)
                    nc.vector.copy_predicated(
                        out=vL3, mask=bc3(maskb, w0, w1, OUT),
                        data=yb[:, w0 * OUT:w1 * OUT].rearrange(
                            "p (w f) -> p w f", f=OUT))

                    if not last:
                        bblk = wk.tile([P, nw * OUT], bf16, tag="bblk")
                        nc.vector.tensor_tensor(
                            out=bblk[:].rearrange("p (w f) -> p w f", f=OUT),
                            in0=vL3, in1=db, op=mybir.AluOpType.mult)
                        nc.sync.dma_start(
                            out=bnL[(ti + 1) % 2][(w0 - wU) * P:(w1 - wU) * P, :]
                                .rearrange("(w p) f -> p w f", p=P),
                            in_=bblk[:].rearrange("p (w f) -> p w f", w=nw))
                    else:
                        oblk = wk.tile([P, nw * OUT], f32, tag="ofin")
                        for a in range(nw):
                            fuse_window(w0 + a, oblk, a)
                        nc.sync.dma_start(
                            out=out_sh[w0 * P:w1 * P, :]
                                .rearrange("(w p) f -> p w f", p=P),
                            in_=oblk[:].rearrange("p (w f) -> p w f", w=nw))

                stage_agg(layL, tab4_aps(tabL[ti], NC * RU + 8), None, ohL_d,
                          OUT, cb_lbl)
                if not last:
                    nc.gpsimd.collective_compute(
                        "AllGather", bass.mybir.AluOpType.bypass,
                        replica_groups=[list(range(NC))],
                        ins=[bnL[(ti + 1) % 2][:, :].opt()],
                        outs=[tabL[(ti + 1) % 2][0:NC * RU, :].opt()])

    nc.compile()
    return nc


_CACHE = {}


def kernel(x, y, edge_index, deep_walk_emb, label_input_mask,
           W_gcn0, b_gcn0, W_gcn1, b_gcn1, W_label, b_label, W_fuse, b_fuse):
    import concourse.bass_utils as bass_utils
    import ml_dtypes

    n_nodes = x.shape[0]
    ei = np.asarray(edge_index, dtype=np.int64)
    meta = _preprocess(ei, np.asarray(label_input_mask), n_nodes)
    core_of, local_of = meta["core_of"], meta["local_of"]
    layP, layL, layC = meta["layP"], meta["layL"], meta["layC"]

    nonzero_b = (bool(np.any(np.asarray(b_gcn0))),
                 bool(np.any(np.asarray(b_gcn1))),
                 bool(np.any(np.asarray(b_label))),
                 bool(np.any(np.asarray(b_fuse))))
    if nonzero_b[2]:
        raise NotImplementedError("nonzero label bias not wired")

    key = ("v2", n_nodes, ei.shape[1], nonzero_b, meta["wU"],
           layP["totcols"], layL["totcols"], layC["totcols"],
           layP["ncols"].tobytes(), layL["ncols"].tobytes(),
           layC["ncols"].tobytes())
    if key not in _CACHE:
        _CACHE[key] = _build(meta, nonzero_b)
    nc = _CACHE[key]

    x_s = _shard_nodes(x, core_of, local_of, IN_DIM)
    y_s = _shard_nodes(y, core_of, local_of, OUT)
    dw_s = _shard_nodes(deep_walk_emb, core_of, local_of, DW)
    mk_s = _shard_nodes(np.asarray(label_input_mask, np.int8)[:, None],
                        core_of, local_of, 1, dtype=np.int8)
    dg_s = np.zeros((NC, NPAD, 1), np.int32)
    dg_s[core_of, local_of, 0] = meta["deg"].astype(np.int32)

    bmax = max(HID, OUT) * NUM_LBL
    b_all = np.zeros((4, bmax), np.float32)
    b_all[0, :HID] = np.asarray(b_gcn0, np.float32)
    b_all[1, :HID] = np.asarray(b_gcn1, np.float32)
    b_all[2, :OUT * NUM_LBL] = np.asarray(b_label, np.float32).reshape(-1)
    b_all[3, :OUT] = np.asarray(b_fuse, np.float32)

    Wl_flat = np.asarray(W_label, np.float32).reshape(NUM_LBL * OUT, OUT)
    idxP128 = np.tile(layP["idx16"], (1, 8, 1))
    idxL128 = np.tile(layL["idx16"], (1, 8, 1))
    idxC128 = np.tile(layC["idx16"], (1, 8, 1))

    def onehots(dpart):
        ncc, pp_, tcc = dpart.shape
        out = np.empty((ncc, pp_, tcc * 128), ml_dtypes.bfloat16)
        ar = np.arange(128, dtype=np.float32)
        for c in range(ncc):
            out[c] = (dpart[c][:, :, None] == ar).reshape(
                pp_, tcc * 128).astype(ml_dtypes.bfloat16)
        return out

    ohP = onehots(layP["dpart"])
    ohL = onehots(layL["dpart"])
    ohC = onehots(layC["dpart"])

    in_maps = []
    for c in range(NC):
        in_maps.append({
            "x_sh": x_s[c], "y_sh": y_s[c], "dw_sh": dw_s[c],
            "mask_sh": mk_s[c], "deg_sh": dg_s[c],
            "idxP_d": idxP128[c], "idxL_d": idxL128[c], "idxC_d": idxC128[c],
            "ohP_d": ohP[c], "ohL_d": ohL[c], "ohC_d": ohC[c],
            "W0": np.asarray(W_gcn0, np.float32),
            "W1": np.asarray(W_gcn1, np.float32),
            "Wl": Wl_flat,
            "Wf": np.asarray(W_fuse, np.float32),
            "b_all": b_all,
        })
    res = bass_utils.run_bass_kernel_spmd(nc, in_maps, core_ids=list(range(NC)))
    out = np.empty((n_nodes, OUT), np.float32)
    for c in range(NC):
        sel = core_of == np.int64(c)
        out[sel] = res.results[c]["out_sh"][local_of[sel]]
    return out
